# revision 4
# baseline (speedup 1.0000x reference)
"""Context-gating kernel for trn2, 8-core SPMD, self-contained.

reference:
    h_norm = rmsnorm(hidden, w_h); e_norm = rmsnorm(engrams, w_e)
    scores = einsum('bsh,bskh->bsk', h_norm, e_norm) / sqrt(H)
    alphas = sigmoid(scores + bias)
    contribution = einsum('bsk,bskh->bsh', alphas, engrams)
    returns (contribution, alphas)

Math used here (no e_norm materialization):
    dots[t,k]  = sum_h (h[t]*w_h*w_e*SCALE*rsqrt(mean h^2+eps))[h] * e[t,k,h]
    scores     = dots * rsqrt(mean_h e^2 + eps)
    alphas     = sigmoid(scores + bias)
    contribution[t] = sum_k alphas[t,k] * e[t,k,:]   (PE diag-matmul, PSUM acc)

Sharding: B*S = 4096 tokens split 512/core across 8 cores; params replicated.
"""
import numpy as np

import bass_rust
import concourse.bass as bass
import concourse.mybir as mybir
import concourse.tile as tile
from concourse.bass_utils import run_bass_kernel_spmd

N_CORES = 8
B, S, K, H = 2, 2048, 8, 2048
T = B * S                  # 4096 tokens
TPC = T // N_CORES         # 512 tokens per core
P = 128                    # partitions / tokens per group
G = TPC // P               # 4 groups per core
EPS = 1e-6
SCALE = 1.0 / np.sqrt(H)
HCH = 512                  # matmul N chunk (one PSUM bank, fp32)

f32 = mybir.dt.float32

TRACE = False
TRACE_KW = {}
LAST_RESULT = None

_counter = [0]


def _fix_multiwait(nc):
    """This container's walrus accepts at most ONE sem wait per instruction;
    split extra waits onto single-wait NoOps on the same engine."""
    n = 0
    for f in nc.m.functions:
        for b in f.blocks:
            out, changed = [], False
            for ins in b.instructions:
                si = ins.sync_info
                waits = list(si.on_wait) if si and si.on_wait else []
                if len(waits) > 1:
                    changed = True
                    n += 1
                    for w in waits[:-1]:
                        _counter[0] += 1
                        nop = mybir.InstNoOp(
                            name=f"wsplit_{_counter[0]}", ins=[], outs=[])
                        nop.engine = ins.engine
                        nop.sync_info = bass_rust.SyncInfo(
                            on_wait=[w], on_update=[])
                        out.append(nop)
                    ins.sync_info = bass_rust.SyncInfo(
                        on_wait=[waits[-1]],
                        on_update=list(si.on_update or []))
                out.append(ins)
            if changed:
                b.instructions = out
    return n


def _build():
    nc = bass.Bass()
    d_h = nc.dram_tensor("h", [TPC, H], f32, kind="ExternalInput")
    d_e = nc.dram_tensor("e", [TPC, K, H], f32, kind="ExternalInput")
    d_w = nc.dram_tensor("wprod", [P, H], f32, kind="ExternalInput")
    d_bias = nc.dram_tensor("biasb", [P, 1], f32, kind="ExternalInput")
    d_I = nc.dram_tensor("ident", [P, P], f32, kind="ExternalInput")
    d_contr = nc.dram_tensor("contr", [TPC, H], f32, kind="ExternalOutput")
    d_alphas = nc.dram_tensor("alphas", [TPC, K], f32, kind="ExternalOutput")

    Sq = mybir.ActivationFunctionType.Square
    Sqrt = mybir.ActivationFunctionType.Sqrt
    Sig = mybir.ActivationFunctionType.Sigmoid
    mult = mybir.AluOpType.mult

    with tile.TileContext(nc) as tc:
        with (
            tc.tile_pool(name="const", bufs=1) as cpool,
            tc.tile_pool(name="hp", bufs=2) as hpool,
            tc.tile_pool(name="ghp", bufs=2) as ghpool,
            tc.tile_pool(name="ep", bufs=10) as epool,
            tc.tile_pool(name="scr", bufs=2) as scrpool,
            tc.tile_pool(name="st", bufs=2) as stpool,
            tc.tile_pool(name="outp", bufs=2) as outpool,
            tc.tile_pool(name="ps", bufs=2, space="PSUM") as pspool,
        ):
            t_w = cpool.tile([P, H], f32)
            t_bias = cpool.tile([P, 1], f32)
            t_I = cpool.tile([P, P], f32)
            t_eps = cpool.tile([P, 1], f32)
            nc.sync.dma_start(out=t_w[:], in_=d_w[:])
            nc.sync.dma_start(out=t_bias[:], in_=d_bias[:])
            nc.sync.dma_start(out=t_I[:], in_=d_I[:])
            nc.vector.memset(t_eps[:], EPS)

            for g in range(G):
                rows = slice(g * P, (g + 1) * P)
                t_h = hpool.tile([P, H], f32)
                nc.sync.dma_start(out=t_h[:], in_=d_h[rows, :])

                # s_h = rsqrt(mean(h^2)+eps)  (folded wprod carries SCALE)
                st = stpool.tile([P, 6], f32, tag="stats")
                scr_h = scrpool.tile([P, H], f32, tag="scr")
                nc.scalar.activation(out=scr_h[:], in_=t_h[:], func=Sq,
                                     accum_out=st[:, 0:1])
                nc.scalar.activation(out=st[:, 1:2], in_=st[:, 0:1],
                                     func=Sqrt, bias=t_eps[:, 0:1],
                                     scale=1.0 / H)
                nc.vector.reciprocal(out=st[:, 2:3], in_=st[:, 1:2])

                # gh = (h * s_h) * (w_h*w_e*SCALE)
                t_gh = ghpool.tile([P, H], f32)
                nc.vector.scalar_tensor_tensor(
                    out=t_gh[:], in0=t_h[:], scalar=st[:, 2:3], in1=t_w[:],
                    op0=mult, op1=mult)

                t_ve = stpool.tile([P, K], f32, tag="ve")
                t_dots = stpool.tile([P, K], f32, tag="dots")
                t_es = []
                for k in range(K):
                    t_e = epool.tile([P, H], f32)
                    t_es.append(t_e)
                    nc.sync.dma_start(out=t_e[:], in_=d_e[rows, k, :])
                    scr_e = scrpool.tile([P, H], f32, tag="scr")
                    nc.scalar.activation(out=scr_e[:], in_=t_e[:], func=Sq,
                                         accum_out=t_ve[:, k:k + 1])
                    scr_d = scrpool.tile([P, H], f32, tag="scrd")
                    nc.vector.scalar_tensor_tensor(
                        out=scr_d[:], in0=t_e[:], scalar=1.0, in1=t_gh[:],
                        op0=mult, op1=mult,
                        accum_out=t_dots[:, k:k + 1])

                # alphas = sigmoid(dots * rsqrt(mean e^2 + eps) + bias)
                t_sv = stpool.tile([P, K], f32, tag="sv")
                nc.scalar.activation(out=t_sv[:], in_=t_ve[:], func=Sqrt,
                                     bias=t_eps[:, 0:1], scale=1.0 / H)
                nc.vector.reciprocal(out=t_sv[:], in_=t_sv[:])
                t_sc = stpool.tile([P, K], f32, tag="sc")
                nc.vector.tensor_mul(t_sc[:], t_dots[:], t_sv[:])
                t_al = stpool.tile([P, K], f32, tag="al")
                nc.scalar.activation(out=t_al[:], in_=t_sc[:], func=Sig,
                                     bias=t_bias[:, 0:1], scale=1.0)
                nc.sync.dma_start(out=d_alphas[rows, :], in_=t_al[:])

                # contribution = sum_k diag(alpha_k) @ e_k   (PSUM acc)
                ps = pspool.tile([P, H], f32)
                for k in range(K):
                    t_diag = stpool.tile([P, P], f32, tag="diag")
                    nc.vector.tensor_scalar_mul(
                        t_diag[:], t_I[:], t_al[:, k:k + 1])
                    for c in range(0, H, HCH):
                        nc.tensor.matmul(
                            out=ps[:, c:c + HCH], lhsT=t_diag[:],
                            rhs=t_es[k][:, c:c + HCH],
                            start=(k == 0), stop=(k == K - 1))
                t_out = outpool.tile([P, H], f32)
                nc.scalar.copy(out=t_out[:], in_=ps[:])
                nc.sync.dma_start(out=d_contr[rows, :], in_=t_out[:])

    _fix_multiwait(nc)
    return nc


_NC = None


def kernel(hidden_states, engrams, w_h, w_e, bias):
    global _NC, LAST_RESULT
    if _NC is None:
        _NC = _build()
    hidden = np.ascontiguousarray(
        np.asarray(hidden_states, dtype=np.float32).reshape(T, H))
    eng = np.ascontiguousarray(
        np.asarray(engrams, dtype=np.float32).reshape(T, K, H))
    wprod = (np.asarray(w_h, dtype=np.float32)
             * np.asarray(w_e, dtype=np.float32) * np.float32(SCALE))
    wprod_b = np.ascontiguousarray(
        np.broadcast_to(wprod[None, :], (P, H)).astype(np.float32))
    bias_b = np.full((P, 1), np.float32(np.asarray(bias)), dtype=np.float32)
    ident = np.eye(P, dtype=np.float32)

    in_maps = []
    for c in range(N_CORES):
        rows = slice(c * TPC, (c + 1) * TPC)
        in_maps.append({
            "h": hidden[rows],
            "e": eng[rows],
            "wprod": wprod_b,
            "biasb": bias_b,
            "ident": ident,
        })

    res = run_bass_kernel_spmd(
        _NC, in_maps, list(range(N_CORES)),
        trace=TRACE, trace_cores=list(range(N_CORES)) if TRACE else None,
        **TRACE_KW)
    LAST_RESULT = res

    contribution = np.concatenate(
        [res.results[c]["contr"] for c in range(N_CORES)], axis=0
    ).reshape(B, S, H)
    alphas = np.concatenate(
        [res.results[c]["alphas"] for c in range(N_CORES)], axis=0
    ).reshape(B, S, K)
    return (contribution, alphas)
